# revision 19
# baseline (speedup 1.0000x reference)
"""DCellLinear batched-GEMM kernel for 8 TRN2 NeuronCores.

Problem: y[s] = x[s] @ W[s].T + b[s] for 4096 independent subsystems,
x[s]: [64, 128], W[s]: [128, 128] (torch Linear layout), b[s]: [128].
Output: concat over s -> [262144, 128] float32.

Strategy (pure data parallel, no collectives):
  - Shard the subsystem axis across 8 cores (512 subsystems/core).
  - Per core, process chunks of CH=32 subsystems:
      * SWDGE cast-DMA loads f32->bf16 into a partition-contiguous layout
        (each SBUF partition holds consecutive DRAM rows -> 1 descriptor
        per partition, line-rate DMA).
      * PE transposes (matmuls against the identity) produce x^T / W^T
        tiles with d_in on partitions. Groups of t_group transposes share
        one PSUM bank (one accumulation group) and one evacuation copy.
      * Main matmuls in bf16: one [128, 256] matmul per subsystem PAIR --
        both subsystems' x^T columns fill the 128-wide stationary array,
        both W^T blocks stream; the two diagonal [64, 128] blocks are the
        useful outputs. Two pairs share one PSUM bank in one group.
      * Bias added with one K=1 rank-1 matmul per bank (ones row x the 4
        subsystems' bias rows) accumulating into the same PSUM group.
      * DVE/ACT strided copies extract the diagonal blocks PSUM->SBUF;
        HWDGE stores bf16 output (upcast to f32 on the host).
  - Compute dtype is bf16 (inputs and accumulation stay f32); the output
    is stored bf16 (halves write traffic) and upcast to f32 on the host.
    Groups of t_group=8 transposes (transpose-mode matmuls, bf16 PSUM:
    8x128 bf16 = exactly one 2 KiB bank, so the accumulation group's
    has_written clear covers the whole tile) share one PSUM bank and one
    evacuation copy -- 6 instead of 24 copies per chunk.
"""

import numpy as np
from contextlib import ExitStack

import concourse.bass as bass
import concourse.mybir as mybir
from concourse.tile import TileContext
from concourse.bass_utils import run_bass_kernel_spmd

try:
    from concourse._compat import axon_active
except ImportError:                      # older trees
    def axon_active():
        return False

# Problem shape (hardcoded per harness contract).
N_SUB, BATCH, D_IN, D_OUT = 4096, 64, 128, 128
N_CORES = 8
S_CORE = N_SUB // N_CORES          # 512 subsystems per core
CH = 32                            # subsystems per chunk
NCHUNK = S_CORE // CH              # 16 chunks
XR = CH * BATCH                    # 2048 x/y rows per chunk
WR = CH * D_OUT                    # 4096 W rows per chunk
XPP = XR // 128                    # 16 x-rows per SBUF partition
WPP = WR // 128                    # 32 W-rows per SBUF partition
QS = BATCH // XPP                  # 4 q-blocks per subsystem (x side)
QW = D_OUT // WPP                  # 4 q-blocks per subsystem (W side)

COMPUTE_DTYPE = mybir.dt.bfloat16
OUT_DTYPE = mybir.dt.bfloat16


def build_nc(cdt=COMPUTE_DTYPE, passes=1, sbuf_bufs=2, psum_t_bufs=4,
             psum_y_bufs=4, t_act_mod=2, extract_split=True,
             split_waits=True, real_mm_transpose=False, out_dt=None,
             ch=None, t_group=8):
    """passes>1 repeats the whole workload inside one NEFF (same inputs,
    same outputs) -- used only for timing via slope; results identical.
    t_act_mod=k routes every k-th transpose-copy pair to ACT (0 = all DVE);
    extract_split routes the A-half extraction to DVE instead of ACT.
    split_waits applies the walrus 1-wait workaround (disable for CoreSim's
    race detector, which chokes on the rewritten tail drain)."""
    if out_dt is None:
        out_dt = OUT_DTYPE
    if ch is None:
        ch = CH
    nchunk = S_CORE // ch
    xr = ch * BATCH                # x/y rows per chunk
    wr = ch * D_OUT                # W rows per chunk
    xpp = xr // 128                # x-rows per SBUF partition
    wpp = wr // 128                # W-rows per SBUF partition
    nc = bass.Bass()
    x_in = nc.declare_dram_parameter(
        "x", [S_CORE * BATCH, D_IN], mybir.dt.float32, isOutput=False)
    w_in = nc.declare_dram_parameter(
        "W", [S_CORE * D_OUT, D_IN], mybir.dt.float32, isOutput=False)
    b_in = nc.declare_dram_parameter(
        "b", [S_CORE, D_OUT], mybir.dt.float32, isOutput=False)
    id_in = nc.declare_dram_parameter(
        "ident", [128, 128], mybir.dt.float32, isOutput=False)
    ones_in = nc.declare_dram_parameter(
        "ones1", [1, 128], mybir.dt.float32, isOutput=False)
    y_out = nc.declare_dram_parameter(
        "out", [S_CORE * BATCH, D_OUT], out_dt, isOutput=True)

    cast_load = cdt != mybir.dt.float32
    # Casting f32->bf16 during DMA requires SWDGE (gpsimd); plain f32 loads
    # can use the faster HWDGE (sync) path.
    ld = nc.gpsimd if cast_load else nc.sync

    with TileContext(nc) as tc, ExitStack() as ctx:
        consts = ctx.enter_context(tc.tile_pool(name="consts", bufs=1))
        xn_pool = ctx.enter_context(tc.tile_pool(name="xn_pool", bufs=sbuf_bufs))
        wn_pool = ctx.enter_context(tc.tile_pool(name="wn_pool", bufs=sbuf_bufs))
        bc_pool = ctx.enter_context(tc.tile_pool(name="bc_pool", bufs=sbuf_bufs))
        xt_pool = ctx.enter_context(tc.tile_pool(name="xt_pool", bufs=sbuf_bufs))
        wt_pool = ctx.enter_context(tc.tile_pool(name="wt_pool", bufs=sbuf_bufs))
        yc_pool = ctx.enter_context(tc.tile_pool(name="yc_pool", bufs=sbuf_bufs))
        pt_pool = ctx.enter_context(tc.tile_pool(name="pt_pool", bufs=psum_t_bufs, space="PSUM"))
        py_pool = ctx.enter_context(tc.tile_pool(name="py_pool", bufs=psum_y_bufs, space="PSUM"))

        ident = consts.tile([128, 128], cdt)
        ld.dma_start(out=ident, in_=id_in[:, :])
        ones1 = consts.tile([1, 128], cdt)
        ld.dma_start(out=ones1, in_=ones_in[:, :])

        def transpose_group(dst2, src3, t, rpp, tg):
            """Transpose src3[:, t+j, :] for j in [0, tg) through one PSUM
            tile (single accumulation group), then copy all out in one op
            into the row-indexed layout dst2[i, r] = row(r)[i], where
            row r lives at source partition r // rpp, slot r % rpp.
            real_mm_transpose uses a regular matmul against the identity
            (out = in.T @ I in f32 PSUM, cast back to cdt during the copy):
            identical math, but counts as PE-busy for the HAM clock gate
            and is FWL-eligible."""
            pdt = mybir.dt.float32 if real_mm_transpose else cdt
            ps = pt_pool.tile([128, tg, 128], pdt)
            for j in range(tg):
                nc.tensor.matmul(ps[:, j, :], src3[:, t + j, :], ident,
                                 is_transpose=not real_mm_transpose,
                                 start=(j == 0), stop=(j == tg - 1))
            # dst positions r = rpp*p + (t+j): strided free AP. All transpose
            # copies stay on DVE so downstream matmuls wait on one engine.
            dst = dst2.rearrange("i (p t) -> i t p", t=rpp)[:, t:t + tg, :]
            if t_act_mod and (t // tg) % t_act_mod == t_act_mod - 1:
                nc.scalar.copy(dst, ps)
            else:
                nc.vector.tensor_copy(dst, ps)

        for c in [c for _ in range(passes) for c in range(nchunk)]:
            # xn[p, r, i] = x_row(c*xr + xpp*p + r)[i]: per-partition data is
            # contiguous in DRAM (xpp rows of 512B).
            xn = xn_pool.tile([128, xpp, 128], cdt)
            ld.dma_start(
                out=xn,
                in_=x_in[c * xr:(c + 1) * xr, :].rearrange("(p r) i -> p r i", p=128))
            wn = wn_pool.tile([128, wpp, 128], cdt)
            ld.dma_start(
                out=wn,
                in_=w_in[c * wr:(c + 1) * wr, :].rearrange("(p r) i -> p r i", p=128))
            # bc[0, s*128 + o] = b[c*ch + s, o]
            bc = bc_pool.tile([1, ch * 128], cdt)
            b_rows = b_in[:, :].rearrange("(c s) o -> c (s o)", s=ch)
            ld.dma_start(out=bc, in_=b_rows[c:c + 1, :])

            # xt[i, r] = x_row(c*xr + r)[i]  (row-indexed transpose of x)
            xt = xt_pool.tile([128, xr], cdt)
            for t in range(0, xpp, t_group):
                transpose_group(xt, xn, t, xpp, t_group)
            # wt[i, r] = W_row(c*wr + r)[i]
            wt = wt_pool.tile([128, wr], cdt)
            for t in range(0, wpp, t_group):
                transpose_group(wt, wn, t, wpp, t_group)

            # yc[p, g, o] = y row (c*xr + 128g + p), col o
            yc = yc_pool.tile([128, ch // 2, 128], out_dt)
            for h in range(ch // 4):      # 2 pairs (4 subsystems) per bank
                yp = py_pool.tile([128, 2, 2, 128], mybir.dt.float32)  # 1 bank
                for j in range(2):
                    g = 2 * h + j         # pair index within chunk
                    # lhsT: pair rows 128g..128g+127 -> M=128, natural order.
                    lhs = xt[:, 128 * g:128 * g + 128]
                    # rhs: pair W-rows 256g..256g+255 -> N=256, natural order.
                    rhs = wt[:, 256 * g:256 * g + 256]
                    nc.tensor.matmul(yp[:, j, :, :], lhs, rhs,
                                     start=(j == 0), stop=False)
                # Bias for the 4 subsystems in this bank in one rank-1 MM.
                nc.tensor.matmul(yp[:, :, :, :], ones1,
                                 bc[0:1, h * 512:(h + 1) * 512],
                                 start=False, stop=True)
                # Diagonal extraction: pair j's useful blocks are
                # yp[0:64, j, 0, :] (subsystem 2g) and yp[64:128, j, 1, :].
                # Both on ACT: PSUM slot release then depends on one engine.
                if extract_split:
                    nc.vector.tensor_copy(yc[0:64, 2 * h:2 * h + 2, :],
                                          yp[0:64, :, 0, :])
                else:
                    nc.scalar.copy(yc[0:64, 2 * h:2 * h + 2, :],
                                   yp[0:64, :, 0, :])
                nc.scalar.copy(yc[64:128, 2 * h:2 * h + 2, :],
                               yp[64:128, :, 1, :])

            nc.sync.dma_start(
                out=y_out[c * xr:(c + 1) * xr, :].rearrange("(g p) o -> p g o", p=128),
                in_=yc)

    if split_waits:
        _split_excess_waits(nc)
    return nc


# Walrus codegen allows only one sync-wait slot on engine-compute
# instructions (e.g. "Matmult: Too many sync wait commands"), but Tile's
# scheduler can emit several. Hoist the extras onto same-engine NoOps
# inserted just before the instruction: the NX sequencer processes waits
# in order before dispatch, so ordering semantics are preserved.
_WAIT_EXEMPT = {
    "InstCall", "InstUnconditionalBranch",
    "InstEventSemaphore", "InstISA", "InstHalt",
}


def _split_excess_waits(nc, max_waits=1):
    import concourse.mybir as mybir_
    k = 0
    for f in nc.m.functions:
        for blk in f.blocks:
            out = []
            changed = False
            for inst in blk.instructions:
                si = getattr(inst, "sync_info", None)
                if (si is not None and si.on_wait and len(si.on_wait) > max_waits
                        and type(inst).__name__ not in _WAIT_EXEMPT):
                    waits = list(si.on_wait)
                    for w in waits[:-max_waits]:
                        nop = mybir_.InstNoOp(name=f"I-nopw{k}")
                        k += 1
                        nop.engine = inst.engine
                        nop.sync_info = mybir_.SyncInfo(on_wait=[w], on_update=[])
                        out.append(nop)
                    inst.sync_info = mybir_.SyncInfo(
                        on_wait=waits[-max_waits:], on_update=list(si.on_update))
                    changed = True
                out.append(inst)
            if changed:
                blk.instructions = out


_CACHE = {}


def _get_nc():
    if "nc" not in _CACHE:
        _CACHE["nc"] = build_nc()
    return _CACHE["nc"]


def _constants():
    ident = np.eye(128, dtype=np.float32)
    ones1 = np.ones((1, 128), dtype=np.float32)
    return ident, ones1


def _in_maps(x, W, b):
    ident, ones1 = _constants()
    maps = []
    for i in range(N_CORES):
        sl = slice(i * S_CORE, (i + 1) * S_CORE)
        maps.append({
            "x": np.ascontiguousarray(x[sl]).reshape(S_CORE * BATCH, D_IN),
            "W": np.ascontiguousarray(W[sl]).reshape(S_CORE * D_OUT, D_IN),
            "b": np.ascontiguousarray(b[sl]),
            "ident": ident,
            "ones1": ones1,
        })
    return maps


def _run(x, W, b, trace=False, **kw):
    x = np.asarray(x, dtype=np.float32)
    W = np.asarray(W, dtype=np.float32)
    b = np.asarray(b, dtype=np.float32)
    res = run_bass_kernel_spmd(
        _get_nc(), _in_maps(x, W, b), core_ids=list(range(N_CORES)),
        trace=trace, **kw)
    y = np.concatenate([np.asarray(res.results[i]["out"]).astype(np.float32)
                        for i in range(N_CORES)], axis=0)
    return y, res


def _prepare_pjrt():
    """Build the jitted SPMD executable and the static device buffers ONCE.

    The library path (run_bass_via_pjrt) re-traces a fresh jax.jit, rebuilds
    a ~536 MB concatenated host array, and re-uploads the zero output
    buffers on EVERY call. Caching the executable and the call-invariant
    buffers (identity, ones, output zeros) leaves only the unavoidable
    per-call input upload + result fetch."""
    import jax
    from jax.sharding import Mesh, PartitionSpec, NamedSharding
    try:
        from jax.experimental.shard_map import shard_map
    except ImportError:
        from jax.shard_map import shard_map
    from concourse.bass2jax import _bass_exec_p, install_neuronx_cc_hook

    nc = _get_nc()
    if nc.partition_id_tensor is not None:
        raise RuntimeError("fast path assumes no partition_id tensor")
    install_neuronx_cc_hook()
    in_names, out_names, out_avals, zero_outs = [], [], [], []
    for alloc in nc.m.functions[0].allocations:
        if not isinstance(alloc, mybir.MemoryLocationSet):
            continue
        name = alloc.memorylocations[0].name
        if alloc.kind == "ExternalInput":
            in_names.append(name)
        elif alloc.kind == "ExternalOutput":
            shape = tuple(alloc.tensor_shape)
            dtype = mybir.dt.np(alloc.dtype)
            out_names.append(name)
            out_avals.append(jax.core.ShapedArray(shape, dtype))
            zero_outs.append(np.zeros(shape, dtype))
    all_names = in_names + out_names

    def _body(*args):
        outs = _bass_exec_p.bind(
            *args, out_avals=tuple(out_avals), in_names=tuple(all_names),
            out_names=tuple(out_names), lowering_input_output_aliases=(),
            sim_require_finite=True, sim_require_nnan=True, nc=nc)
        return tuple(outs)

    devices = jax.devices("axon")[:N_CORES]
    mesh = Mesh(np.asarray(devices), ("core",))
    nin = len(in_names) + len(out_names)
    fn = jax.jit(shard_map(_body, mesh=mesh,
                           in_specs=(PartitionSpec("core"),) * nin,
                           out_specs=(PartitionSpec("core"),) * len(out_names),
                           check_rep=False), keep_unused=True)
    sharding = NamedSharding(mesh, PartitionSpec("core"))
    ident, ones1 = _constants()
    static = {
        "ident": jax.device_put(np.tile(ident, (N_CORES, 1)), sharding),
        "ones1": jax.device_put(np.tile(ones1, (N_CORES, 1)), sharding),
    }
    zero_dev = [
        jax.device_put(np.zeros((N_CORES * z.shape[0], *z.shape[1:]), z.dtype),
                       sharding)
        for z in zero_outs]
    return {"fn": fn, "sharding": sharding, "in_names": in_names,
            "static": static, "zero_dev": zero_dev}


def _run_axon_fast(x, W, b):
    import jax
    if "pjrt" not in _CACHE:
        _CACHE["pjrt"] = _prepare_pjrt()
    P = _CACHE["pjrt"]
    # The core-sharded concatenation of per-core slices IS the full array:
    # no host-side copies, just reshaped views.
    full = {
        "x": np.ascontiguousarray(x.reshape(N_SUB * BATCH, D_IN)),
        "W": np.ascontiguousarray(W.reshape(N_SUB * D_OUT, D_IN)),
        "b": np.ascontiguousarray(b),
    }
    args = [P["static"][n] if n in P["static"]
            else jax.device_put(full[n], P["sharding"])
            for n in P["in_names"]]
    args += P["zero_dev"]
    outs = P["fn"](*args)
    return np.asarray(outs[0]).astype(np.float32)


def kernel(x, W, b):
    x = np.asarray(x, dtype=np.float32)
    W = np.asarray(W, dtype=np.float32)
    b = np.asarray(b, dtype=np.float32)
    if axon_active():
        try:
            return _run_axon_fast(x, W, b)
        except Exception:
            _CACHE.pop("pjrt", None)     # fall back to the library path
    y, _ = _run(x, W, b, trace=False)
    return y

